# revision 1
# baseline (speedup 1.0000x reference)
"""Trainium2 Bass kernel for nn_Attribution (sparse local-window attention).

Data-parallel over batch n=8 -> one batch element per NeuronCore.

Per-core computation (c_in=256, ch=128, 64x64 image):
    h    = W1 @ x + b1
    corr = 5x5 local window correlation of h (zero padded), /sqrt(128)
    attn = softmax over the 25 window entries
    samp = sum_k attn_k * shift_k(h)
    gate = sigmoid(relu(W2 @ h + b2)) = 0.5 + 0.5*relu(tanh((z+b2)/2))
    out  = Wout @ (gate * samp) + bout

Layout: positions flattened row-major with 2 zero-pad rows top/bottom
(68 rows x 64 = 4352 positions = 34 chunks of 128).  Scores are computed
"born transposed" (keys of one chunk on partitions, queries on the free
axis): for key chunk c the queries of all subs needing it are contiguous,
so one matmul (n<=384) produces all scores of that chunk.  Out-of-window
entries are killed by a {0,1} band mask after exp; out-of-image x
neighbors are accounted by a denominator correction D (they contribute
exp(0)=1 in the zero-padded reference).  Softmax normalization is
commuted through the output convolution; reciprocals are computed with a
bit-hack seed + 3 Newton iterations on VectorE in a compact (32,128)
layout reached via a 16KB DMA reshape.
"""
import sys

sys.path.insert(0, "/opt/trn_rl_repo")

import numpy as np
import ml_dtypes

import concourse.bass as bass
import concourse.mybir as mybir
import concourse.tile as tile
from concourse import bacc
from concourse.bass_utils import run_bass_kernel_spmd
from concourse.masks import make_identity

F32 = mybir.dt.float32
BF16 = mybir.dt.bfloat16
I32 = mybir.dt.int32
F32R = mybir.dt.float32r
AF = mybir.ActivationFunctionType
ALU = mybir.AluOpType

N, CIN, CH, H, W = 8, 256, 128, 64, 64
HW = H * W                      # 4096
RAD = 2
KROWS = H + 2 * RAD             # 68 padded rows
PADPOS = KROWS * W              # 4352
NCHUNK = PADPOS // 128          # 34 key chunks (2 rows each)
NSUB = H // 2                   # 32 query subs (128 queries each)
SCALE = 1.0 / np.sqrt(np.float32(CH))
RECIP_MAGIC = 0x7EF127EA


def _build_mask_and_D():
    """maskC: (128, 384) {0,1}; col 128*a+q is the score of key (chunk c,
    pos p) vs query q of sub s = c-2+a.  Valid iff |2-2a + p//64 - q//64|
    <= 2 and |p%64 - q%64| <= 2.   D: (32,128) = 5*cnt(qx)."""
    m = np.zeros((128, 384), dtype=np.float32)
    for a in range(3):
        for p in range(128):
            for q in range(128):
                dy = 2 - 2 * a + p // 64 - q // 64
                if abs(dy) <= RAD and abs(p % 64 - q % 64) <= RAD:
                    m[p, 128 * a + q] = 1.0
    maskC = m.astype(ml_dtypes.bfloat16)

    cnt = np.array([sum(1 for dx in range(-RAD, RAD + 1) if not 0 <= qx + dx < W)
                    for qx in range(W)], dtype=np.float32)
    Drow = 5.0 * np.concatenate([cnt, cnt])
    D = np.zeros((64, 128), np.float32)
    D[0:16] = Drow[None, :]
    D[32:48] = Drow[None, :]
    return maskC, D


def build_nc(repeat=1, sim_safe=False):
    nc = bacc.Bacc("TRN2", target_bir_lowering=False, debug=False, num_devices=8)

    x_d = nc.declare_dram_parameter("x", [CIN, HW], BF16, isOutput=False)
    w1t_d = nc.declare_dram_parameter("W1T", [CIN, CH], BF16, isOutput=False)
    b1_d = nc.declare_dram_parameter("b1", [CH, 1], F32, isOutput=False)
    w2t_d = nc.declare_dram_parameter("W2T", [CH, CH], BF16, isOutput=False)
    b2h_d = nc.declare_dram_parameter("b2h", [CH, 1], F32, isOutput=False)
    wot_d = nc.declare_dram_parameter("WoutT", [CH, CIN], BF16, isOutput=False)
    bout_d = nc.declare_dram_parameter("bout2", [CH, 2], F32, isOutput=False)
    boutr_d = nc.declare_dram_parameter("boutrow", [1, CIN], BF16, isOutput=False)
    mask_d = nc.declare_dram_parameter("maskC", [128, 384], BF16, isOutput=False)
    dvec_d = nc.declare_dram_parameter("Dvec", [64, 128], F32, isOutput=False)
    ident_d = nc.declare_dram_parameter("ident", [128, 128], BF16, isOutput=False)
    onescol_d = nc.declare_dram_parameter("onescol_c", [128, 1], BF16, isOutput=False)
    ones1_d = nc.declare_dram_parameter("ones1_c", [1, 512], BF16, isOutput=False)
    out_d = nc.declare_dram_parameter("out", [CIN, HW], F32, isOutput=True)

    with tile.TileContext(nc) as tc:
        with (
            tc.tile_pool(name="per", bufs=1) as per,
            tc.tile_pool(name="xb", bufs=4) as xbp,
            tc.tile_pool(name="sm", bufs=4) as smp,
            tc.tile_pool(name="ot", bufs=4) as otp,
            tc.tile_pool(name="psc", bufs=2, space="PSUM") as psc,   # score chunks
            tc.tile_pool(name="pss", bufs=3, space="PSUM") as pss,   # generic 1-bank
            tc.tile_pool(name="psd", bufs=1, space="PSUM") as psd,   # denominators
        ):
            hpad = per.tile([128, PADPOS], BF16, tag="hpad")
            hT = per.tile([128, PADPOS], BF16, tag="hT")
            attnm = per.tile([128, NCHUNK * 512], BF16, tag="attnm")
            Pg = per.tile([128, HW], BF16, tag="Pg")
            attr = per.tile([128, HW], BF16, tag="attr")
            denrow = per.tile([1, HW], BF16, tag="denrow")
            recrow = per.tile([1, HW], BF16, tag="recrow")

            w1t0 = per.tile([128, CH], BF16, tag="w1t0")
            w1t1 = per.tile([128, CH], BF16, tag="w1t1")
            w2t = per.tile([128, CH], BF16, tag="w2t")
            wot = per.tile([128, CIN], BF16, tag="wot")
            b1 = per.tile([CH, 1], F32, tag="b1")
            b2h = per.tile([CH, 1], F32, tag="b2h")
            bout = per.tile([CH, 2], F32, tag="bout")
            boutrow = per.tile([1, CIN], BF16, tag="boutrow")
            maskC = per.tile([128, 384], BF16, tag="maskC")
            maskC2g = per.tile([128, 896], BF16, tag="maskC2g")
            dvec = per.tile([64, 128], F32, tag="dvec")
            onescol = per.tile([128, 1], BF16, tag="onescol")
            ones1 = per.tile([1, 512], BF16, tag="ones1")
            ident = per.tile([128, 128], BF16, tag="ident")
            denq = per.tile([64, 128], F32, tag="denq")
            denqb = per.tile([64, 128], BF16, tag="denqb")
            newt = per.tile([64, 128], F32, tag="newt")
            ntmp = per.tile([64, 128], F32, tag="ntmp")

            nc.sync.dma_start(w1t0[:], w1t_d[0:128, :])
            nc.sync.dma_start(w1t1[:], w1t_d[128:256, :])
            nc.scalar.dma_start(b1[:], b1_d[:])
            nc.vector.memset(hpad[:, 0:128], 0.0)
            nc.vector.memset(maskC2g[:, 384:512], 0.0)
            nc.vector.memset(hpad[:, PADPOS - 128:PADPOS], 0.0)

            for _rep in range(repeat):
                # ---- P1: conv1 (x cast to bf16 during DMA)
                xts = []
                for t in range(4):
                    x0 = xbp.tile([128, 1024], BF16, tag="x0")
                    x1 = xbp.tile([128, 1024], BF16, tag="x1")
                    cs = slice(1024 * t, 1024 * (t + 1))
                    nc.sync.dma_start(x0[:], x_d[0:128, cs])
                    nc.scalar.dma_start(x1[:], x_d[128:256, cs])
                    xts.append((x0, x1))
                nc.scalar.dma_start(ident[:], ident_d[:])
                nc.scalar.dma_start(maskC[:], mask_d[:])
                nc.scalar.dma_start(maskC2g[:, 0:384], mask_d[:])
                nc.scalar.dma_start(maskC2g[:, 512:896], mask_d[:])
                nc.scalar.dma_start(onescol[:], onescol_d[:])
                nc.scalar.dma_start(w2t[:], w2t_d[:])
                nc.scalar.dma_start(b2h[:], b2h_d[:])
                nc.sync.dma_start(dvec[:], dvec_d[:])
                nc.sync.dma_start(ones1[:], ones1_d[:])
                nc.sync.dma_start(wot[:], wot_d[:])
                nc.sync.dma_start(bout[:], bout_d[:])
                nc.sync.dma_start(boutrow[:], boutr_d[:])
                for t in range(4):
                    x0, x1 = xts[t]
                    for u in range(2):
                        ps = pss.tile([128, 512], F32, tag="ps")
                        sl = slice(512 * u, 512 * (u + 1))
                        nc.tensor.matmul(ps[:], w1t0[:], x0[:, sl], start=True, stop=False)
                        nc.tensor.matmul(ps[:], w1t1[:], x1[:, sl], start=False, stop=True)
                        o = 128 + 1024 * t + 512 * u
                        nc.vector.tensor_scalar(
                            out=hpad[:, o:o + 512], in0=ps[:],
                            scalar1=b1[:], scalar2=None, op0=ALU.add)

                # ---- P2: hT via PE transposes (bf16 psum), evac on DVE/ACT
                for c4 in range(9):
                    pt = pss.tile([128, 512], BF16, tag="ps", name=f"pt{c4}")
                    n4 = min(4, NCHUNK - 4 * c4)
                    for k4 in range(n4):
                        c = 4 * c4 + k4
                        nc.tensor.transpose(pt[:, 128 * k4:128 * (k4 + 1)],
                                            hpad[:, 128 * c:128 * (c + 1)], ident[:])
                    nc.vector.tensor_copy(hT[:, 512 * c4:512 * c4 + 128 * n4],
                                          pt[:, 0:128 * n4])

                # ---- P3a: scores/exp/mask per chunk pair + inline denominators
                dn = None
                for cp in range(NCHUNK // 2):
                    sc = psc.tile([128, 1024], F32, tag="sc", name=f"sc{cp}")
                    spans = []
                    for ci in range(2):
                        c = 2 * cp + ci
                        lo, hi = max(0, c - 2), min(NSUB - 1, c)
                        nsubs = hi - lo + 1
                        alo = lo - (c - 2)
                        spans.append((alo, alo + nsubs))
                        dst = sc[:, 512 * ci + 128 * alo:512 * ci + 128 * (alo + nsubs)]
                        nc.tensor.matmul(
                            dst, hpad[:, 128 * c:128 * (c + 1)],
                            hpad[:, 128 * (lo + 1):128 * (hi + 2)],
                            start=True, stop=True)
                    if not sim_safe and spans == [(0, 3), (0, 3)]:
                        asl = attnm[:, 1024 * cp:1024 * cp + 896]
                        nc.scalar.activation(asl, sc[:, 0:896], AF.Exp,
                                             scale=float(SCALE))
                        nc.vector.tensor_tensor(out=asl, in0=asl,
                                                in1=maskC2g[:], op=ALU.mult)
                    else:
                        for ci, (a0, a1) in enumerate(spans):
                            ss = slice(512 * ci + 128 * a0, 512 * ci + 128 * a1)
                            asl = attnm[:, 1024 * cp + ss.start:1024 * cp + ss.stop]
                            nc.scalar.activation(asl, sc[:, ss], AF.Exp,
                                                 scale=float(SCALE))
                            nc.vector.tensor_tensor(
                                out=asl, in0=asl,
                                in1=maskC[:, 128 * a0:128 * a1], op=ALU.mult)
                    # denominators: 4-row super-sub S ready once chunk 2S+3 done
                    for S in ({cp - 1} if cp >= 1 else set()):
                        if not 0 <= S < 16:
                            continue
                        if S % 2 == 0 or dn is None:
                            dn = psd.tile([1, 512], F32, tag="dn", name=f"dn{S}")
                        base = 256 * (S % 2)
                        for h2 in range(2):
                            s2 = 2 * S + h2
                            for j in range(3):
                                c = s2 + j
                                aa = 2 - j
                                nc.tensor.matmul(
                                    dn[0:1, base + 128 * h2:base + 128 * (h2 + 1)],
                                    onescol[:],
                                    attnm[:, 512 * c + 128 * aa:512 * c + 128 * (aa + 1)],
                                    start=(j == 0), stop=(j == 2))
                        if S % 2 == 1:
                            g = S // 2
                            nc.vector.tensor_copy(
                                denrow[0:1, 512 * g:512 * (g + 1)], dn[0:1, :])

                # ---- P3c: conv2 + tanh -> Pg = relu(tanh)+1 (gate pre recip)
                for t in range(8):
                    pz = pss.tile([128, 512], F32, tag="ps")
                    nc.tensor.matmul(pz[:], w2t[:],
                                     hpad[:, 128 + 512 * t:128 + 512 * (t + 1)],
                                     start=True, stop=True)
                    tg = smp.tile([128, 512], BF16, tag="tg")
                    nc.scalar.activation(tg[:], pz[:], AF.Tanh, scale=0.5, bias=b2h[:])
                    nc.vector.tensor_scalar(
                        out=Pg[:, 512 * t:512 * (t + 1)], in0=tg[:],
                        scalar1=0.0, scalar2=1.0, op0=ALU.max, op1=ALU.add)

                # ---- P3e(i): sample matmuls for groups 0-3 (PE runway)
                sp_tiles = {}

                def emit_sample_mms(g8):
                    pool = psc if g8 % 2 == 0 else pss
                    tg_ = "sc" if pool is psc else "ps"
                    sp = pool.tile([128, 512], F32, tag=tg_, name=f"sp{g8}")
                    sp_tiles[g8] = sp
                    for a4 in range(4):
                        s4 = 4 * g8 + a4
                        for j in range(3):
                            c = s4 + j
                            aa = 2 - j
                            nc.tensor.matmul(
                                sp[:, 128 * a4:128 * (a4 + 1)],
                                hT[:, 128 * c:128 * (c + 1)],
                                attnm[:, 512 * c + 128 * aa:512 * c + 128 * (aa + 1)],
                                start=(j == 0), stop=(j == 2))

                def emit_attr_convout(g8):
                    sp = sp_tiles.pop(g8)
                    gsl = slice(512 * g8, 512 * (g8 + 1))
                    nc.vector.tensor_tensor(out=attr[:, gsl], in0=sp[:],
                                            in1=Pg[:, gsl], op=ALU.mult)
                    for oc in range(2):
                        po = pss.tile([128, 512], F32, tag="ps", name=f"po{g8}_{oc}")
                        if oc == 1:
                            nc.tensor.matmul(po[:], boutrow[0:1, 128:256],
                                             ones1[0:1, :], start=True, stop=False)
                            nc.tensor.matmul(po[:], wot[:, 128:256], attr[:, gsl],
                                             start=False, stop=True)
                        else:
                            nc.tensor.matmul(po[:], wot[:, 0:128], attr[:, gsl],
                                             start=True, stop=True)
                        ot = otp.tile([128, 512], F32, tag="ot")
                        if oc == 1:
                            nc.scalar.activation(ot[:], po[:], AF.Copy)
                        else:
                            nc.vector.tensor_scalar(out=ot[:], in0=po[:],
                                                    scalar1=bout[:, 0:1],
                                                    scalar2=None, op0=ALU.add)
                        nc.sync.dma_start(out_d[128 * oc:128 * (oc + 1), gsl], ot[:])


                def emit_chain_half(hh):
                    hs = slice(32 * hh, 32 * hh + 16)
                    rs = slice(2048 * hh, 2048 * (hh + 1))
                    nc.sync.dma_start(
                        denqb[hs, :],
                        denrow[0:1, rs].rearrange("o (s f) -> o s f", s=16))
                    nc.vector.tensor_copy(denq[hs, :], denqb[hs, :])
                    nc.vector.tensor_tensor(out=denq[hs, :], in0=denq[hs, :],
                                            in1=dvec[hs, :], op=ALU.add)
                    nc.vector.tensor_scalar(out=newt[hs, :].bitcast(I32),
                                            in0=denq[hs, :].bitcast(I32),
                                            scalar1=0, scalar2=None, op0=ALU.bitwise_not)
                    nc.vector.tensor_scalar(out=newt[hs, :].bitcast(I32),
                                            in0=newt[hs, :].bitcast(I32),
                                            scalar1=RECIP_MAGIC + 1, scalar2=None, op0=ALU.add)
                    for _ in range(3):
                        nc.vector.tensor_tensor(out=ntmp[hs, :], in0=denq[hs, :],
                                                in1=newt[hs, :], op=ALU.mult)
                        nc.vector.tensor_scalar(out=ntmp[hs, :], in0=ntmp[hs, :],
                                                scalar1=-1.0, scalar2=2.0,
                                                op0=ALU.mult, op1=ALU.add)
                        nc.vector.tensor_tensor(out=newt[hs, :], in0=newt[hs, :],
                                                in1=ntmp[hs, :], op=ALU.mult)
                    nc.vector.tensor_scalar(out=denqb[hs, :], in0=newt[hs, :],
                                            scalar1=0.5, scalar2=None, op0=ALU.mult)
                    nc.sync.dma_start(
                        recrow[0:1, rs].rearrange("o (s f) -> o s f", s=16),
                        denqb[hs, :])
                    for t in range(4 * hh, 4 * hh + 4):
                        if t % 2 == 0:
                            pb = psd.tile([128, 512], F32, tag="dn", name=f"pb{t}")
                        else:
                            pb = pss.tile([128, 512], F32, tag="ps", name=f"pb{t}")
                        nc.tensor.matmul(pb[0:128, :], ones1[0:1, 0:128],
                                         recrow[0:1, 512 * t:512 * (t + 1)],
                                         start=True, stop=True)
                        sl = slice(512 * t, 512 * (t + 1))
                        nc.vector.tensor_tensor(out=Pg[:, sl], in0=Pg[:, sl],
                                                in1=pb[0:128, :], op=ALU.mult)

                emit_chain_half(0)
                for g8 in range(4):
                    emit_sample_mms(g8)
                for g8 in range(4):
                    emit_attr_convout(g8)
                for g8 in range(4, 6):
                    emit_sample_mms(g8)
                emit_chain_half(1)
                for g8 in range(6, 8):
                    emit_sample_mms(g8)
                for g8 in range(4, 8):
                    emit_attr_convout(g8)


    return nc


def _prep_inputs(x, W1, b1, W2, b2, Wout, bout):
    maskC, D = _build_mask_and_D()
    bf = ml_dtypes.bfloat16
    common = {
        "W1T": np.ascontiguousarray(W1.T).astype(bf),
        "b1": np.asarray(b1, np.float32).reshape(CH, 1),
        "W2T": np.ascontiguousarray(W2.T).astype(bf),
        "b2h": (0.5 * np.asarray(b2, np.float32)).reshape(CH, 1),
        "WoutT": np.ascontiguousarray(Wout.T).astype(bf),
        "bout2": np.ascontiguousarray(np.asarray(bout, np.float32).reshape(2, CH).T),
        "boutrow": np.asarray(bout, np.float32).reshape(1, CIN).astype(bf),
        "maskC": maskC,
        "Dvec": D,
        "ident": np.eye(128, dtype=np.float32).astype(bf),
        "onescol_c": np.ones((128, 1), np.float32).astype(bf),
        "ones1_c": np.ones((1, 512), np.float32).astype(bf),
    }
    in_maps = []
    for i in range(N):
        m = dict(common)
        m["x"] = np.ascontiguousarray(
            np.asarray(x[i], np.float32).reshape(CIN, HW)).astype(bf)
        in_maps.append(m)
    return in_maps


_CACHED = {}


def kernel(x, W1, b1, W2, b2, Wout, bout):
    if "nc" not in _CACHED:
        nc = build_nc()
        nc.finalize()
        _CACHED["nc"] = nc
    nc = _CACHED["nc"]
    in_maps = _prep_inputs(x, W1, b1, W2, b2, Wout, bout)
    res = run_bass_kernel_spmd(nc, in_maps, core_ids=list(range(N)))
    out = np.stack([res.results[i]["out"].reshape(CIN, H, W) for i in range(N)])
    return out.astype(np.float32)



# revision 16
# speedup vs baseline: 1.1926x; 1.1926x over previous
"""Trainium2 Bass kernel for nn_Attribution (sparse local-window attention).

Data-parallel over batch n=8 -> one batch element per NeuronCore.

Per-core computation (c_in=256, ch=128, 64x64 image):
    h    = W1 @ x + b1
    corr = 5x5 local window correlation of h (zero padded), /sqrt(128)
    attn = softmax over the 25 window entries
    samp = sum_k attn_k * shift_k(h)
    gate = sigmoid(relu(W2 @ h + b2))
    out  = Wout @ (gate * samp) + bout

Layout: positions flattened row-major with 2 zero-pad rows top/bottom
(68 rows x 64 = 4352 positions = 34 chunks of 128).  Scores are "born
transposed" (keys of chunk c on psum partitions, queries on free axis).
exp'd+masked scores live c-major in attnm: chunk c block a (query sub
s=c-2+a) at cols 384c+128a.

Key structure vs a naive implementation:
  - hT (position-major h) via 4 DMA-transpose instructions, not PE.
  - denominators are computed REPLICATED across partitions by using an
    all-2.0 (128,128) stationary, so softmax normalization is a single
    DVE divide; the zero-pad correction 2*D is preloaded into the psum
    accumulator by a K=1 matmul.  Factor 2 matches Pg = 2*gate =
    1+relu(tanh(z/2+b2/2)).
  - out-conv bias: oc0 added by DVE during psum evac; oc1 preloaded
    into psum (K=1 matmul) and evac'd by ACT copy.  Output is written
    bf16 and widened to f32 on host.
  - per-chunk pipeline with LAG so PE never waits on ACT exp / DVE
    mask: [score c] ... [sample c-3, den c-3] interleaved.
"""
import sys

sys.path.insert(0, "/opt/trn_rl_repo")

import numpy as np
import ml_dtypes

import concourse.bass as bass
import concourse.mybir as mybir
import concourse.tile as tile
from concourse import bacc
from concourse.bass_utils import run_bass_kernel_spmd

F32 = mybir.dt.float32
BF16 = mybir.dt.bfloat16
AF = mybir.ActivationFunctionType
ALU = mybir.AluOpType

N, CIN, CH, H, W = 8, 256, 128, 64, 64
HW = H * W                      # 4096
RAD = 2
KROWS = H + 2 * RAD             # 68 padded rows
PADPOS = KROWS * W              # 4352
NCHUNK = PADPOS // 128          # 34 key chunks (2 rows each)
NSUB = H // 2                   # 32 query subs (128 queries each)
SCALE = 1.0 / np.sqrt(np.float32(CH))
LAG = 2

# CB (bf16 const block) column layout
CB_W1T0 = 0
CB_W1T1 = 128
CB_W2T = 256
CB_WOT = 384          # (128, 256)
CB_MASK = 640         # (128, 384)
CB_TWOS = 1024        # (128, 128) of 2.0
CB_D512 = 1152        # row 0: (1, 512) pad-correction D
CB_BOUT1 = 1664       # row 0: (1, 128) bout[128:256]
CB_ONES512 = 1792     # row 0: (1, 512) of 1.0
CB_COLS = 2304


def _build_mask_and_D():
    """maskC: (128, 384) {0,1}; col 128*a+q is the score of key (chunk c,
    pos p) vs query q of sub s = c-2+a.  Valid iff |2-2a + p//64 - q//64|
    <= 2 and |p%64 - q%64| <= 2.   D: (512,) = 5*cnt(qx) tiled (the number
    of window slots per query that fall off the row ends; each contributes
    exp(0)=1 to the reference softmax denominator)."""
    m = np.zeros((128, 384), dtype=np.float32)
    for a in range(3):
        for p in range(128):
            for q in range(128):
                dy = 2 - 2 * a + p // 64 - q // 64
                if abs(dy) <= RAD and abs(p % 64 - q % 64) <= RAD:
                    m[p, 128 * a + q] = 1.0
    cnt = np.array([sum(1 for dx in range(-RAD, RAD + 1) if not 0 <= qx + dx < W)
                    for qx in range(W)], dtype=np.float32)
    Drow = 5.0 * np.concatenate([cnt, cnt, cnt, cnt])   # (256,) -> tile to 512
    D512 = np.concatenate([Drow, Drow])[:512]
    return m, D512


def build_nc(repeat=1, sim_safe=False, dbg=False):
    nc = bacc.Bacc("TRN2", target_bir_lowering=False, debug=False, num_devices=8)

    x_d = nc.declare_dram_parameter("x", [CIN, HW], BF16, isOutput=False)
    cb_d = nc.declare_dram_parameter("CB", [128, CB_COLS], BF16, isOutput=False)
    bf_d = nc.declare_dram_parameter("BF", [128, 4], F32, isOutput=False)
    out_d = nc.declare_dram_parameter("out", [CIN, HW], BF16, isOutput=True)
    if dbg:
        dbg_d = {nm: nc.declare_dram_parameter(f"dbg_{nm}", shp, dt, isOutput=True)
                 for nm, shp, dt in [
                     ("hpad", [128, PADPOS], BF16), ("hT", [128, PADPOS], BF16),
                     ("attnm", [128, NCHUNK * 384], BF16), ("Pg", [128, HW], BF16),
                     ("rden", [128, HW], F32), ("spg", [128, HW], BF16)]}

    with tile.TileContext(nc) as tc:
        with (
            tc.tile_pool(name="per", bufs=1) as per,
            tc.tile_pool(name="psc", bufs=3, space="PSUM") as psc,   # scores
            tc.tile_pool(name="psa", bufs=2, space="PSUM") as psa,   # conv2 / sample
            tc.tile_pool(name="pdn", bufs=2, space="PSUM") as pdn,   # conv1 / den
            tc.tile_pool(name="pou", bufs=1, space="PSUM") as pou,   # convout
        ):
            xsb0 = per.tile([128, HW], BF16, tag="xsb0")
            xsb1 = per.tile([128, HW], BF16, tag="xsb1")
            hpad = per.tile([128, PADPOS], BF16, tag="hpad")
            hT = per.tile([128, PADPOS], BF16, tag="hT")
            attnm = per.tile([128, NCHUNK * 384], BF16, tag="attnm")
            Pg = per.tile([128, HW], BF16, tag="Pg")
            spg = per.tile([128, HW], BF16, tag="spg")
            outsb = per.tile([128, 2 * HW], BF16, tag="outsb")
            rden = per.tile([128, HW], F32, tag="rden")
            cb = per.tile([128, CB_COLS], BF16, tag="cb")
            bfc = per.tile([128, 4], F32, tag="bfc")

            for _rep in range(repeat):
                # ---- input + const DMAs (4KB-contiguous runs, both queues)
                nc.sync.dma_start(xsb0[:, 0:2048], x_d[0:128, 0:2048])
                nc.sync.dma_start(xsb0[:, 2048:4096], x_d[0:128, 2048:4096])
                nc.scalar.dma_start(cb[:], cb_d[:])
                nc.scalar.dma_start(bfc[:], bf_d[:])
                nc.scalar.dma_start(xsb1[:, 0:2048], x_d[128:256, 0:2048])
                nc.scalar.dma_start(xsb1[:, 2048:4096], x_d[128:256, 2048:4096])
                nc.gpsimd.memset(hpad[:, 0:128], 0.0)
                nc.gpsimd.memset(hpad[:, PADPOS - 128:PADPOS], 0.0)

                # ---- P1: conv1 (+bias on DVE), conv2 gate lagging 2 tiles
                def emit_conv1(t):
                    pc = pdn.tile([128, 512], F32, tag="pc")
                    sl = slice(512 * t, 512 * (t + 1))
                    nc.tensor.matmul(pc[:], cb[:, CB_W1T0:CB_W1T0 + 128],
                                     xsb0[:, sl], start=True, stop=False)
                    nc.tensor.matmul(pc[:], cb[:, CB_W1T1:CB_W1T1 + 128],
                                     xsb1[:, sl], start=False, stop=True)
                    nc.vector.tensor_scalar(
                        out=hpad[:, 128 + 512 * t:128 + 512 * (t + 1)], in0=pc[:],
                        scalar1=bfc[:, 0:1], scalar2=None, op0=ALU.add)

                def emit_conv2(t):
                    pz = psa.tile([128, 512], F32, tag="pz")
                    hsl = slice(128 + 512 * t, 128 + 512 * (t + 1))
                    sl = slice(512 * t, 512 * (t + 1))
                    nc.tensor.matmul(pz[:], cb[:, CB_W2T:CB_W2T + 128],
                                     hpad[:, hsl], start=True, stop=True)
                    # Pg = 1 + relu(tanh(z/2 + b2/2)) = 2*sigmoid(relu(z))
                    nc.scalar.activation(Pg[:, sl], pz[:], AF.Tanh,
                                         scale=0.5, bias=bfc[:, 1:2])
                    nc.vector.tensor_scalar(out=Pg[:, sl], in0=Pg[:, sl],
                                            scalar1=0.0, scalar2=1.0,
                                            op0=ALU.max, op1=ALU.add)

                for t in range(8):
                    emit_conv1(t)
                    if t >= 2:
                        emit_conv2(t - 2)
                emit_conv2(6)
                emit_conv2(7)

                # ---- hT via DMA transpose (4 quarters of 9/9/9/7 chunks)
                for q in range(4):
                    c0, c1 = 9 * q, min(NCHUNK, 9 * (q + 1))
                    cols = slice(128 * c0, 128 * c1)
                    nc.sync.dma_start(
                        hT[:, cols].rearrange("p (c k) -> p c k", k=128),
                        hpad[:, cols], transpose=True)

                # ---- P2: per-chunk pipeline
                sc_meta = {}
                samp_tiles = {}
                den_tiles = {}
                po_tiles = {}

                def emit_score(c):
                    lo, hi = max(0, c - 2), min(NSUB - 1, c)
                    alo, ahi = lo - (c - 2), hi - (c - 2)
                    sc = psc.tile([128, 512], F32, tag="sc")
                    sc_meta[c] = (alo, ahi)
                    psl = slice(128 * alo, 128 * (ahi + 1))
                    nc.tensor.matmul(sc[:, psl],
                                     hpad[:, 128 * c:128 * (c + 1)],
                                     hpad[:, 128 * (lo + 1):128 * (hi + 2)],
                                     start=True, stop=True)
                    asl = attnm[:, 384 * c + 128 * alo:384 * c + 128 * (ahi + 1)]
                    nc.scalar.activation(asl, sc[:, psl], AF.Exp, scale=float(SCALE))
                    nc.vector.tensor_tensor(
                        out=asl, in0=asl,
                        in1=cb[:, CB_MASK + 128 * alo:CB_MASK + 128 * (ahi + 1)],
                        op=ALU.mult)

                def emit_sampden(c):
                    alo, ahi = sc_meta[c]
                    if c % 4 == 0 and c < 32:
                        # den tile for supersub G=c//4: first use is chunk 4G
                        G = c // 4
                        den_tiles[G] = pdn.tile([128, 512], F32, tag="pc",
                                                name=f"dn{G}")
                        # preload 2*D (pad correction) replicated on all rows
                        nc.tensor.matmul(den_tiles[G][:],
                                         cb[0:1, CB_TWOS:CB_TWOS + 128],
                                         cb[0:1, CB_D512:CB_D512 + 512],
                                         start=True, stop=False,
                                         skip_group_check=True)
                    if c >= 2 and (c - 2) % 4 == 0:
                        # sample tile for supersub (c-2)//4: first sub fires now
                        G = (c - 2) // 4
                        samp_tiles[G] = psa.tile([128, 512], F32, tag="pz",
                                                 name=f"sp{G}")
                    # sample for sub s=c-2: all 3 blocks exist now; 3
                    # back-to-back MMs = one accumulation group per bank
                    # (a new psum `start` wipes open groups in its bank)
                    if c >= 2:
                        s = c - 2
                        G, j = s // 4, s % 4
                        for a in range(3):
                            cc_ = s + 2 - a
                            blk = attnm[:, 384 * cc_ + 128 * a:
                                        384 * cc_ + 128 * (a + 1)]
                            nc.tensor.matmul(
                                samp_tiles[G][:, 128 * j:128 * (j + 1)],
                                hT[:, 128 * cc_:128 * (cc_ + 1)], blk,
                                start=(a == 0), stop=(a == 2),
                                skip_group_check=True)
                    for a in range(ahi, alo - 1, -1):
                        s = c - 2 + a
                        G, j = s // 4, s % 4
                        blk = attnm[:, 384 * c + 128 * a:384 * c + 128 * (a + 1)]
                        nc.tensor.matmul(
                            den_tiles[G][:, 128 * j:128 * (j + 1)],
                            cb[:, CB_TWOS:CB_TWOS + 128], blk,
                            start=False, stop=(a == 0), skip_group_check=True)

                def emit_finishA(G):
                    qsl = slice(512 * G, 512 * (G + 1))
                    sp_ = samp_tiles.pop(G)
                    den_ = den_tiles.pop(G)
                    nc.vector.reciprocal_approx_fast(out=rden[:, qsl], in_=den_[:])
                    nc.vector.tensor_tensor(out=spg[:, qsl], in0=sp_[:],
                                            in1=Pg[:, qsl], op=ALU.mult)
                    nc.vector.tensor_tensor(out=spg[:, qsl], in0=spg[:, qsl],
                                            in1=rden[:, qsl], op=ALU.mult)
                    po = pou.tile([128, 512], F32, tag="po", name=f"po{G}a")
                    po_tiles[G] = po
                    nc.tensor.matmul(po[:], cb[:, CB_WOT:CB_WOT + 128],
                                     spg[:, qsl], start=True, stop=True)
                    nc.vector.tensor_scalar(
                        out=outsb[:, qsl], in0=po[:],
                        scalar1=bfc[:, 2:3], scalar2=None, op0=ALU.add)
                    if G % 2 == 1:
                        # oc0 rows for supersub pair (G-1, G): 2KB runs
                        psl = slice(512 * (G - 1), 512 * (G + 1))
                        nc.sync.dma_start(out_d[0:128, psl], outsb[:, psl])

                def emit_finishB(G):
                    qsl = slice(512 * G, 512 * (G + 1))
                    po_tiles.pop(G)
                    po2 = pou.tile([128, 512], F32, tag="po", name=f"po{G}b")
                    nc.tensor.matmul(po2[:], cb[0:1, CB_BOUT1:CB_BOUT1 + 128],
                                     cb[0:1, CB_ONES512:CB_ONES512 + 512],
                                     start=True, stop=False, skip_group_check=True)
                    nc.tensor.matmul(po2[:], cb[:, CB_WOT + 128:CB_WOT + 256],
                                     spg[:, qsl], start=False, stop=True,
                                     skip_group_check=True)
                    nc.scalar.activation(outsb[:, 4096 + 512 * G:4096 + 512 * (G + 1)],
                                         po2[:], AF.Copy)
                    if G % 2 == 1:
                        psl = slice(512 * (G - 1), 512 * (G + 1))
                        nc.scalar.dma_start(out_d[128:256, psl],
                                            outsb[:, 4096 + psl.start:4096 + psl.stop])

                for cc in range(NCHUNK + LAG + 1):
                    if cc < NCHUNK:
                        emit_score(cc)
                    d = cc - LAG
                    if 0 <= d < NCHUNK:
                        emit_sampden(d)
                    if d >= 5 and (d - 5) % 4 == 0 and (d - 5) // 4 < 8:
                        emit_finishA((d - 5) // 4)
                    if d >= 6 and (d - 6) % 4 == 0 and (d - 6) // 4 < 8:
                        emit_finishB((d - 6) // 4)

                if dbg:
                    for nm, t in [("hpad", hpad), ("hT", hT), ("attnm", attnm),
                                  ("Pg", Pg), ("rden", rden), ("spg", spg)]:
                        nc.sync.dma_start(dbg_d[nm][:], t[:])

    return nc


def _prep_inputs(x, W1, b1, W2, b2, Wout, bout):
    maskC, D512 = _build_mask_and_D()
    bf = ml_dtypes.bfloat16
    CB = np.zeros((128, CB_COLS), np.float32)
    W1T = np.ascontiguousarray(np.asarray(W1, np.float32).T)   # (256, 128)
    CB[:, CB_W1T0:CB_W1T0 + 128] = W1T[0:128]
    CB[:, CB_W1T1:CB_W1T1 + 128] = W1T[128:256]
    CB[:, CB_W2T:CB_W2T + 128] = np.asarray(W2, np.float32).T
    CB[:, CB_WOT:CB_WOT + 256] = np.asarray(Wout, np.float32).T
    CB[:, CB_MASK:CB_MASK + 384] = maskC
    CB[:, CB_TWOS:CB_TWOS + 128] = 2.0
    CB[0, CB_D512:CB_D512 + 512] = D512
    CB[0, CB_BOUT1:CB_BOUT1 + 128] = np.asarray(bout, np.float32)[128:256]
    CB[0, CB_ONES512:CB_ONES512 + 512] = 1.0
    BF = np.zeros((128, 4), np.float32)
    BF[:, 0] = np.asarray(b1, np.float32)
    BF[:, 1] = 0.5 * np.asarray(b2, np.float32)
    BF[:, 2] = np.asarray(bout, np.float32)[0:128]
    common = {"CB": CB.astype(bf), "BF": BF}
    in_maps = []
    for i in range(N):
        m = dict(common)
        m["x"] = np.ascontiguousarray(
            np.asarray(x[i], np.float32).reshape(CIN, HW)).astype(bf)
        in_maps.append(m)
    return in_maps


_CACHED = {}


def kernel(x, W1, b1, W2, b2, Wout, bout):
    if "nc" not in _CACHED:
        nc = build_nc()
        nc.finalize()
        _CACHED["nc"] = nc
    nc = _CACHED["nc"]
    in_maps = _prep_inputs(x, W1, b1, W2, b2, Wout, bout)
    res = run_bass_kernel_spmd(nc, in_maps, core_ids=list(range(N)))
    out = np.stack([np.asarray(res.results[i]["out"], dtype=np.float32)
                    .reshape(CIN, H, W) for i in range(N)])
    return out
